# revision 16
# baseline (speedup 1.0000x reference)
"""GCN3 (3-layer GCNConv + BatchNorm + ReLU + linear head) on 8 Trainium2
NeuronCores.

Strategy (graph/data parallel, per sharding hint):
  - Nodes are sharded contiguously: core c owns destination nodes
    [c*12500, (c+1)*12500).
  - Per layer: each core computes z = r @ W_eff for its own nodes, the z
    shards are AllGathered (in 4 row-slices, giving 4 contiguous "striped"
    regions of 25000 rows so gather indices fit in int16), then each core
    gathers z rows for its incoming edges with dma_gather and scatter-adds
    them into per-supertile PSUM accumulators via one-hot matmuls
    (P^T[e, d] = norm_e * (dst_e == d), agg^T = msg^T @ P^T).
  - BatchNorm is folded into the next layer's weights:
    BN(r) @ W = r @ (diag(s) W) + 1 t'^T with a rank-1 per-destination
    correction dcoef[d] * t' added in PSUM (dcoef[d] = sum of incoming edge
    norms of d), where s/t come from an AllReduce of per-core stats.
"""

import math

import numpy as np

import concourse.bacc as bacc
import concourse.bass as bass
import concourse.mybir as mybir
import concourse.tile as tile
from concourse import library_config
from concourse.bass_utils import run_bass_kernel_spmd

# ---------------------------------------------------------------------------
# The walrus build in this container rejects the EVENT_SEMAPHORE_RANGE_CLEAR
# raw-ISA instruction Tile emits at context exit ("ISA wrong length").  Skip
# emitting it but keep the semaphore-ID bookkeeping; the kernel runs once per
# NEFF load so the end-of-kernel semaphore clear is not needed.
def _patched_clear(self, sems):
    if not sems:
        return
    sem_nums = [s.num if hasattr(s, "num") else s for s in sems]
    self._state.prepend_free_semaphores(sem_nums)
    for ps in self._tile_sem_poison_stack:
        ps.update(sem_nums)


bass.Bass.clear_and_free_semaphores = _patched_clear

EPS = 1e-5
NCORES = 8
KSLICES = 4  # AllGather row-slices -> striped regions addressable by int16
ST_W = 256  # supertile width (dst nodes per PSUM accumulator)
G_ST = 2  # supertiles per dma_gather call group
PT_B = 8  # P-matrix builds batched per DVE op
P = 128

DEBUG_SKIP = set()  # {"gather", "collectives"}

F32 = mybir.dt.float32
I16 = mybir.dt.int16


class _Cfg:
    def __init__(self, n_nodes, msg="f32"):
        assert n_nodes % (NCORES * KSLICES) == 0
        self.N = n_nodes
        self.shard = n_nodes // NCORES
        self.slice = self.shard // KSLICES
        self.region = self.slice * NCORES
        assert self.region <= 32767
        self.nst = math.ceil(self.shard / ST_W)
        self.last_w = self.shard - (self.nst - 1) * ST_W
        self.ngrp = math.ceil(self.nst / G_ST)
        self.npad = self.nst * ST_W  # padded shard width
        self.ntile = math.ceil(self.npad / P)  # 128-node tiles
        self.msg = msg
        self.msg_dt = F32 if msg == "f32" else mybir.dt.float16
        self.msg_np = np.float32 if msg == "f32" else np.float16


def _pack_idx16(flat_idx):
    """[n] int16 (n % 128 == 0) -> [128, n//16] with idx i at
    (partition i%16, slot i//16), replicated across the 8 groups of 16."""
    a = flat_idx.reshape(-1, 16).T  # [16, W]
    return np.tile(a, (8, 1)).astype(np.int16)


def _preprocess(cfg, x, edge_index, edge_weights):
    """Host-side graph preprocessing: normalization, edge partitioning, and
    the uniform (cross-core identical) gather/scatter schedule."""
    N = cfg.N
    row = np.asarray(edge_index[0], np.int64)
    col = np.asarray(edge_index[1], np.int64)
    ew = np.asarray(edge_weights, np.float32)

    deg = np.bincount(col, weights=ew.astype(np.float64), minlength=N).astype(
        np.float32
    ) + np.float32(1.0)
    dinv = (1.0 / np.sqrt(deg)).astype(np.float32)
    norm = (dinv[row] * ew * dinv[col]).astype(np.float32)

    arange = np.arange(N, dtype=np.int64)
    src = np.concatenate([row, arange])
    dst = np.concatenate([col, arange])
    nv = np.concatenate([norm, dinv * dinv]).astype(np.float32)

    dcoef = np.bincount(dst, weights=nv.astype(np.float64), minlength=N).astype(
        np.float32
    )

    core = dst // cfg.shard
    st = (dst % cfg.shard) // ST_W
    bkt = (src % cfg.shard) // cfg.slice
    sidx = (src // cfg.shard) * cfg.slice + (src % cfg.slice)
    dloc = (dst % cfg.shard) % ST_W

    key = ((core * cfg.nst) + st) * KSLICES + bkt
    order = np.argsort(key, kind="stable")
    sidx_s = sidx[order].astype(np.int16)
    dloc_s = dloc[order].astype(np.float32)
    nv_s = nv[order]

    counts = np.bincount(key, minlength=NCORES * cfg.nst * KSLICES).reshape(
        NCORES, cfg.nst, KSLICES
    )
    starts = np.zeros_like(counts)
    flat = counts.reshape(-1)
    starts.reshape(-1)[1:] = np.cumsum(flat)[:-1]

    # Uniform chunk counts: max over cores per (st, k).
    nck = np.ceil(counts / P).astype(np.int64).max(axis=0)  # [nst, K]

    # Build the call/chunk schedule (identical for every core).
    calls = []  # per (g, k): dict(num_idxs, w16_off, slot_off, chunks=[...])
    w16_off = 0
    slot_off = 0
    chunk_first = {}
    chunk_last = {}
    for s in range(cfg.nst):
        ks = [k for k in range(KSLICES) if nck[s, k] > 0]
        if ks:
            chunk_first[s] = (ks[0], 0)
            chunk_last[s] = (ks[-1], nck[s, ks[-1]] - 1)
    for g in range(cfg.ngrp):
        sts = list(range(g * G_ST, min((g + 1) * G_ST, cfg.nst)))
        for k in range(KSLICES):
            call_slots = int(sum(nck[s, k] for s in sts))
            if call_slots == 0:
                continue
            chunks = []
            loc = 0
            for s in sts:
                for j in range(int(nck[s, k])):
                    chunks.append(
                        dict(
                            st=s,
                            local_slot=loc,
                            gslot=slot_off + loc,
                            start=chunk_first[s] == (k, j),
                            stop=chunk_last[s] == (k, j),
                        )
                    )
                    loc += 1
            calls.append(
                dict(
                    g=g,
                    k=k,
                    slots=call_slots,
                    num_idxs=call_slots * P,
                    w16_off=w16_off,
                    slot_off=slot_off,
                    chunks=chunks,
                )
            )
            w16_off += call_slots * 8
            slot_off += call_slots
    sched = dict(calls=calls, tot_w16=w16_off, tot_slots=slot_off, nck=nck)

    # Per-core packed constant tensors.
    per_core = []
    for c in range(NCORES):
        idx_cols = []
        dst_cols = []
        nrm_cols = []
        for call in calls:
            k = call["k"]
            ns = call["slots"] * P
            fi = np.zeros(ns, np.int16)
            fd = np.zeros(ns, np.float32)
            fn = np.zeros(ns, np.float32)
            loc = 0
            sts = list(range(call["g"] * G_ST, min((call["g"] + 1) * G_ST, cfg.nst)))
            for s in sts:
                cnt = int(counts[c, s, k])
                off = int(starts[c, s, k])
                pos0 = loc * P
                fi[pos0 : pos0 + cnt] = sidx_s[off : off + cnt]
                fd[pos0 : pos0 + cnt] = dloc_s[off : off + cnt]
                fn[pos0 : pos0 + cnt] = nv_s[off : off + cnt]
                loc += int(nck[s, k])
            idx_cols.append(_pack_idx16(fi))
            dst_cols.append(fd.reshape(-1, P).T)
            nrm_cols.append(fn.reshape(-1, P).T)
        xs = np.asarray(x[c * cfg.shard : (c + 1) * cfg.shard], np.float32)
        xt = np.zeros((P, cfg.npad), np.float32)
        xt[:, : cfg.shard] = xs.T
        dc = np.zeros((1, cfg.npad), np.float32)
        dc[0, : cfg.shard] = dcoef[c * cfg.shard : (c + 1) * cfg.shard]
        per_core.append(
            dict(
                xt=xt,
                dcoef=dc,
                idx=np.concatenate(idx_cols, axis=1),
                dstv=np.concatenate(dst_cols, axis=1).astype(np.float32),
                ndst=-np.concatenate(dst_cols, axis=1).astype(np.float32),
                normv=np.concatenate(nrm_cols, axis=1).astype(cfg.msg_np),
            )
        )
    return sched, per_core


def _build_program(cfg, sched):
    nc = bacc.Bacc("TRN2", target_bir_lowering=False, debug=False, num_devices=NCORES, num_swdge_queues=4)
    dt = cfg.msg_dt
    calls = sched["calls"]
    max_slots = max(c["slots"] for c in calls)

    xt_d = nc.dram_tensor("xt", [P, cfg.npad], F32, kind="ExternalInput")
    idx_d = nc.dram_tensor("idx", [P, sched["tot_w16"]], I16, kind="ExternalInput")
    dstv_d = nc.dram_tensor("dstv", [P, sched["tot_slots"]], F32, kind="ExternalInput")
    ndst_d = nc.dram_tensor("ndst", [P, sched["tot_slots"]], F32, kind="ExternalInput")
    normv_d = nc.dram_tensor("normv", [P, sched["tot_slots"]], dt, kind="ExternalInput")
    iota_d = nc.dram_tensor("iota", [P, ST_W], dt, kind="ExternalInput")
    dcoef_d = nc.dram_tensor("dcoef", [1, cfg.npad], F32, kind="ExternalInput")
    w_d = [nc.dram_tensor(f"w{i}", [P, P], F32, kind="ExternalInput") for i in (1, 2, 3)]
    b_d = [nc.dram_tensor(f"b{i}", [P, 1], F32, kind="ExternalInput") for i in (1, 2, 3)]
    ga_d = [nc.dram_tensor(f"ga{i}", [P, 1], F32, kind="ExternalInput") for i in (1, 2, 3)]
    be_d = [nc.dram_tensor(f"be{i}", [P, 1], F32, kind="ExternalInput") for i in (1, 2, 3)]
    wl_d = nc.dram_tensor("wl", [P, 10], F32, kind="ExternalInput")
    blb_d = nc.dram_tensor("blb", [P, 10], F32, kind="ExternalInput")
    out_d = nc.dram_tensor("out", [cfg.shard, 10], F32, kind="ExternalOutput")

    ivn = 1.0 / float(cfg.N)

    with tile.TileContext(nc) as tc:
        with (
            tc.tile_pool(name="const", bufs=1) as cp,
            tc.tile_pool(name="rbuf", bufs=1) as rp,
            tc.tile_pool(name="msgs", bufs=6) as mp,
            tc.tile_pool(name="idxp", bufs=6) as ip,
            tc.tile_pool(name="ptp", bufs=4) as ptp,
            tc.tile_pool(name="zsb", bufs=4) as zp,
            tc.tile_pool(name="outp", bufs=4) as op_,
            tc.tile_pool(name="small", bufs=1) as sm,
            tc.tile_pool(name="psum", bufs=1, space="PSUM") as pp,
            tc.tile_pool(name="dram", bufs=1, space="DRAM") as dp,
        ):
            nc.gpsimd.load_library(library_config.mlp)

            # --- resident constants ---
            rT = rp.tile([P, cfg.npad], F32, tag="rT")  # x^T, later r^T (layer output)
            nc.sync.dma_start(rT[:], xt_d[:])
            dstv = cp.tile([P, sched["tot_slots"]], F32, tag="dstv")
            nc.sync.dma_start(dstv[:], dstv_d[:])
            normv = cp.tile([P, sched["tot_slots"]], dt, tag="normv")
            nc.sync.dma_start(normv[:], normv_d[:])
            ndst = cp.tile([P, sched["tot_slots"]], F32, tag="ndst")
            nc.sync.dma_start(ndst[:], ndst_d[:])
            iota = cp.tile([P, ST_W], dt, tag="iota")
            nc.sync.dma_start(iota[:], iota_d[:])
            dcoef = cp.tile([1, cfg.npad], F32, tag="dcoef")
            nc.sync.dma_start(dcoef[:], dcoef_d[:])
            w_sb, b_sb, ga_sb, be_sb = [], [], [], []
            for i in range(3):
                w = cp.tile([P, P], F32, name=f"w{i}", tag=f"w{i}")
                nc.sync.dma_start(w[:], w_d[i][:])
                w_sb.append(w)
                for lst, d in ((b_sb, b_d), (ga_sb, ga_d), (be_sb, be_d)):
                    t = cp.tile([P, 1], F32, name=f"p{i}_{len(lst)}", tag=f"p{i}_{len(lst)}")
                    nc.sync.dma_start(t[:], d[i][:])
                    lst.append(t)
            wl = cp.tile([P, 10], F32, tag="wl")
            nc.sync.dma_start(wl[:], wl_d[:])
            blb = cp.tile([P, 10], F32, tag="blb")
            nc.sync.dma_start(blb[:], blb_d[:])

            # --- per-layer DRAM buffers (reused across layers) ---
            z_local = dp.tile([cfg.shard, P], dt, name="z_local", tag="z_local")
            z_str_l = [
                [
                    dp.tile(
                        [cfg.region, P], dt,
                        addr_space="Local" if "collectives" in DEBUG_SKIP else "Shared",
                        name=f"z_str{l}_{k}", tag=f"z_str{l}_{k}",
                    )
                    for k in range(KSLICES)
                ]
                for l in range(3)
            ]

            w_rhs = w_sb[0]
            tp_sb = None  # t'^T [1, P] for the rank-1 BN-fold correction

            for layer in range(3):
                z_str = z_str_l[layer]
                # ---- z phase: z = r^T.T @ W_eff, shard-local ----
                for t in range(cfg.ntile):
                    rows = min(P, cfg.shard - t * P)
                    zp_ps = pp.tile([P, P], F32, tag="z", bufs=2, name=f"zp{layer}_{t}")
                    nc.tensor.matmul(
                        out=zp_ps[:],
                        lhsT=rT[:, t * P : (t + 1) * P],
                        rhs=w_rhs[:],
                        start=True,
                        stop=True,
                    )
                    zsb = zp.tile([P, P], dt, tag="zsb", name=f"zs{layer}_{t}")
                    nc.scalar.copy(zsb[:], zp_ps[:])
                    nc.sync.dma_start(
                        z_local[t * P : t * P + rows, :], zsb[:rows, :]
                    )
                # ---- AllGather z (4 row-slices -> striped regions) ----
                for k in range(KSLICES):
                    if "collectives" in DEBUG_SKIP:
                        for rr in range(NCORES):
                            nc.sync.dma_start(
                                z_str[k][rr * cfg.slice : (rr + 1) * cfg.slice, :],
                                z_local[k * cfg.slice : (k + 1) * cfg.slice, :],
                            )
                        continue
                    nc.gpsimd.collective_compute(
                        "AllGather",
                        mybir.AluOpType.bypass,
                        replica_groups=[list(range(NCORES))],
                        ins=[z_local[k * cfg.slice : (k + 1) * cfg.slice, :]],
                        outs=[z_str[k][:]],
                    )
                # ---- gather + one-hot-matmul scatter ----
                agg = {}
                agg_done = set()
                chunk_i = 0
                for call_i, call in enumerate(calls):
                    g, k = call["g"], call["k"]
                    sts = list(range(g * G_ST, min((g + 1) * G_ST, cfg.nst)))
                    for s in sts:
                        if s not in agg:
                            agg[s] = pp.tile(
                                [P, ST_W], F32, tag="agg", bufs=6,
                                name=f"agg{layer}_{s}",
                            )
                    if "gather" in DEBUG_SKIP:
                        for s_ in sts:
                            if s_ in agg_done:
                                continue
                            agg_done.add(s_)
                            nc.tensor.matmul(
                                out=agg[s_][:], lhsT=iota[:, :P], rhs=iota[:],
                                start=True, stop=(layer == 0),
                            )
                        continue
                    w16 = call["slots"] * 8
                    idxt = ip.tile([P, w16], I16, tag="idx", name=f"ix{layer}_{g}_{k}")
                    nc.sync.dma_start(
                        idxt[:], idx_d[:, call["w16_off"] : call["w16_off"] + w16]
                    )
                    msgs = mp.tile(
                        [P, max_slots, P], dt, tag="msgs",
                        name=f"mg{layer}_{g}_{k}",
                    )[:, : call["slots"], :]
                    nc.gpsimd.dma_gather(
                        msgs[:], z_str[k][:], idxt[:],
                        call["num_idxs"], call["num_idxs"], P,
                        single_packet=False,
                        queue_num=call_i % 4,
                    )
                    # fold norm into the gathered messages (also zeroes the
                    # padding slots, keeping the one-hot P rows harmless)
                    nrm3 = normv[:, call["slot_off"] : call["slot_off"] + call["slots"]]
                    nc.vector.tensor_tensor(
                        out=msgs[:],
                        in0=msgs[:],
                        in1=nrm3.rearrange("p (s one) -> p s one", one=1).to_broadcast(
                            [P, call["slots"], P]
                        ),
                        op=mybir.AluOpType.mult,
                    )
                    for ch in call["chunks"]:
                        gs = ch["gslot"]
                        pt = ptp.tile([P, ST_W], dt, tag="pt", name=f"pt{layer}_{chunk_i}")
                        if chunk_i % 3 == 2:
                            tmp = ptp.tile([P, ST_W], dt, tag="acttmp", name=f"at{layer}_{chunk_i}")
                            nc.scalar.activation(
                                tmp[:], iota[:],
                                mybir.ActivationFunctionType.Abs,
                                bias=ndst[:, gs : gs + 1],
                            )
                            nc.scalar.activation(
                                pt[:], tmp[:],
                                mybir.ActivationFunctionType.Relu,
                                scale=-1.0, bias=1.0,
                            )
                        else:
                            nc.vector.tensor_scalar(
                                out=pt[:], in0=iota[:],
                                scalar1=dstv[:, gs : gs + 1], scalar2=None,
                                op0=mybir.AluOpType.is_equal,
                            )
                        stop = ch["stop"] and (layer == 0)
                        nc.tensor.matmul(
                            out=agg[ch["st"]][:],
                            lhsT=msgs[:, ch["local_slot"], :],
                            rhs=pt[:],
                            start=ch["start"],
                            stop=stop,
                        )
                        chunk_i += 1
                # ---- per-supertile epilogue ----
                ssum = sm.tile([P, cfg.nst], F32, name=f"ssum{layer}", tag=f"ssum{layer}")
                ssq = sm.tile([P, cfg.nst], F32, name=f"ssq{layer}", tag=f"ssq{layer}")
                for s in range(cfg.nst):
                    cols = ST_W if s < cfg.nst - 1 else cfg.last_w
                    if layer > 0:
                        nc.tensor.matmul(
                            out=agg[s][:],
                            lhsT=tp_sb[:],
                            rhs=dcoef[0:1, s * ST_W : s * ST_W + ST_W],
                            start=False,
                            stop=True,
                        )
                    if "no_accum" in DEBUG_SKIP:
                        nc.scalar.activation(
                            rT[:, s * ST_W : s * ST_W + cols],
                            agg[s][:, :cols],
                            mybir.ActivationFunctionType.Relu,
                            bias=b_sb[layer][:],
                        )
                        nc.vector.tensor_reduce(
                            ssum[:, s : s + 1],
                            rT[:, s * ST_W : s * ST_W + cols],
                            mybir.AxisListType.X,
                            mybir.AluOpType.add,
                        )
                    else:
                        nc.scalar.activation(
                            rT[:, s * ST_W : s * ST_W + cols],
                            agg[s][:, :cols],
                            mybir.ActivationFunctionType.Relu,
                            bias=b_sb[layer][:],
                            accum_out=ssum[:, s : s + 1],
                        )
                    scr = op_.tile([P, ST_W], dt, tag="scr", name=f"sc{layer}_{s}")
                    nc.scalar.activation(
                        scr[:, :cols],
                        rT[:, s * ST_W : s * ST_W + cols],
                        mybir.ActivationFunctionType.Square,
                        accum_out=ssq[:, s : s + 1],
                    )
                # ---- global BN stats ----
                spk = sm.tile([P, 2], F32, name=f"spk{layer}", tag=f"spk{layer}")
                nc.vector.tensor_reduce(
                    spk[:, 0:1], ssum[:], mybir.AxisListType.X, mybir.AluOpType.add
                )
                nc.vector.tensor_reduce(
                    spk[:, 1:2], ssq[:], mybir.AxisListType.X, mybir.AluOpType.add
                )
                st_in = dp.tile([P, 2], F32, name=f"sti{layer}", tag=f"sti{layer}")
                st_out = dp.tile([P, 2], F32, addr_space="Local" if "collectives" in DEBUG_SKIP else "Shared", name=f"sto{layer}", tag=f"sto{layer}")
                nc.sync.dma_start(st_in[:], spk[:])
                if "collectives" in DEBUG_SKIP:
                    nc.sync.dma_start(st_out[:], st_in[:])
                else:
                    nc.gpsimd.collective_compute(
                        "AllReduce",
                        mybir.AluOpType.add,
                        replica_groups=[list(range(NCORES))],
                        ins=[st_in[:]],
                        outs=[st_out[:]],
                    )
                sg = sm.tile([P, 2], F32, name=f"sg{layer}", tag=f"sg{layer}")
                nc.sync.dma_start(sg[:], st_out[:])
                # s = gamma * rsqrt(var + eps); t = beta - mu * s
                mu = sm.tile([P, 1], F32, name=f"mu{layer}", tag=f"mu{layer}")
                nc.vector.tensor_scalar(
                    out=mu[:], in0=sg[:, 0:1], scalar1=ivn, scalar2=None,
                    op0=mybir.AluOpType.mult,
                )
                var = sm.tile([P, 1], F32, name=f"var{layer}", tag=f"var{layer}")
                nc.vector.tensor_scalar(
                    out=var[:], in0=sg[:, 1:2], scalar1=ivn, scalar2=None,
                    op0=mybir.AluOpType.mult,
                )
                mu2 = sm.tile([P, 1], F32, name=f"mu2{layer}", tag=f"mu2{layer}")
                nc.vector.tensor_tensor(
                    out=mu2[:], in0=mu[:], in1=mu[:], op=mybir.AluOpType.mult
                )
                nc.vector.tensor_tensor(
                    out=var[:], in0=var[:], in1=mu2[:], op=mybir.AluOpType.subtract
                )
                ve = sm.tile([P, 1], F32, name=f"ve{layer}", tag=f"ve{layer}")
                nc.vector.tensor_scalar(
                    out=ve[:], in0=var[:], scalar1=float(EPS), scalar2=None,
                    op0=mybir.AluOpType.add,
                )
                sd = sm.tile([P, 1], F32, name=f"sd{layer}", tag=f"sd{layer}")
                nc.scalar.activation(
                    sd[:], ve[:], mybir.ActivationFunctionType.Sqrt,
                )
                inv = sm.tile([P, 1], F32, name=f"inv{layer}", tag=f"inv{layer}")
                nc.vector.reciprocal(inv[:], sd[:])
                s_t = sm.tile([P, 1], F32, name=f"s{layer}", tag=f"s{layer}")
                nc.vector.tensor_tensor(
                    out=s_t[:], in0=ga_sb[layer][:], in1=inv[:],
                    op=mybir.AluOpType.mult,
                )
                t_t = sm.tile([P, 1], F32, name=f"t{layer}", tag=f"t{layer}")
                nc.vector.tensor_tensor(
                    out=t_t[:], in0=mu[:], in1=s_t[:], op=mybir.AluOpType.mult
                )
                nc.vector.tensor_tensor(
                    out=t_t[:], in0=be_sb[layer][:], in1=t_t[:],
                    op=mybir.AluOpType.subtract,
                )
                if layer < 2:
                    weff = sm.tile([P, P], F32, name=f"weff{layer}", tag=f"weff{layer}")
                    nc.vector.tensor_scalar(
                        out=weff[:], in0=w_sb[layer + 1][:], scalar1=s_t[:, 0:1],
                        scalar2=None, op0=mybir.AluOpType.mult,
                    )
                    w_rhs = weff
                    tp_ps = pp.tile([1, P], F32, tag="z", bufs=2, name=f"tpp{layer}")
                    nc.tensor.matmul(
                        out=tp_ps[:], lhsT=t_t[:], rhs=w_sb[layer + 1][:],
                        start=True, stop=True,
                    )
                    tp_sb = sm.tile([1, P], F32, name=f"tp{layer}", tag=f"tp{layer}")
                    nc.vector.tensor_copy(tp_sb[:], tp_ps[:])
                else:
                    s3, t3 = s_t, t_t

            # ---- classifier: out = (s3*r3 + t3)^T @ Wl + bl ----
            for t in range(cfg.ntile):
                rows = min(P, cfg.shard - t * P)
                h3 = op_.tile([P, P], F32, tag="h3", name=f"h3_{t}")
                nc.vector.tensor_scalar(
                    out=h3[:],
                    in0=rT[:, t * P : (t + 1) * P],
                    scalar1=s3[:, 0:1],
                    scalar2=t3[:, 0:1],
                    op0=mybir.AluOpType.mult,
                    op1=mybir.AluOpType.add,
                )
                po = pp.tile([P, 10], F32, tag="z", bufs=2, name=f"po{t}")
                nc.tensor.matmul(out=po[:], lhsT=h3[:], rhs=wl[:], start=True, stop=True)
                osb = op_.tile([P, 10], F32, tag="osb", name=f"ob{t}")
                nc.vector.tensor_tensor(
                    out=osb[:], in0=po[:], in1=blb[:], op=mybir.AluOpType.add
                )
                nc.sync.dma_start(out_d[t * P : t * P + rows, :], osb[:rows, :])
    nc.compile()
    return nc


_CACHE = {}


def _get_compiled(cfg, x, edge_index, edge_weights, weights):
    sched, per_core = _preprocess(cfg, x, edge_index, edge_weights)
    nc = _build_program(cfg, sched)
    iota = np.tile(np.arange(ST_W, dtype=cfg.msg_np), (P, 1))
    in_maps = []
    for c in range(NCORES):
        pc = per_core[c]
        m = dict(
            xt=pc["xt"],
            idx=pc["idx"],
            dstv=pc["dstv"],
            ndst=pc["ndst"],
            normv=pc["normv"],
            iota=iota,
            dcoef=pc["dcoef"],
            wl=np.asarray(weights["Wl"], np.float32),
            blb=np.tile(np.asarray(weights["bl"], np.float32), (P, 1)),
        )
        for i in (1, 2, 3):
            m[f"w{i}"] = np.asarray(weights[f"W{i}"], np.float32)
            m[f"b{i}"] = np.asarray(weights[f"b{i}"], np.float32).reshape(P, 1)
            m[f"ga{i}"] = np.asarray(weights[f"g{i}"], np.float32).reshape(P, 1)
            m[f"be{i}"] = np.asarray(weights[f"be{i}"], np.float32).reshape(P, 1)
        in_maps.append(m)
    return nc, in_maps


def run(x, edge_index, edge_weights, weights, msg="f32", trace=False):
    cfg = _Cfg(np.asarray(x).shape[0], msg=msg)
    nc, in_maps = _get_compiled(cfg, x, edge_index, edge_weights, weights)
    r = run_bass_kernel_spmd(nc, in_maps, list(range(NCORES)), trace=trace)
    out = np.concatenate([r.results[c]["out"] for c in range(NCORES)], axis=0)
    return out, r


def kernel(
    x,
    edge_index,
    edge_weights,
    W1, b1, g1, be1,
    W2, b2, g2, be2,
    W3, b3, g3, be3,
    Wl, bl,
):
    weights = dict(
        W1=W1, b1=b1, g1=g1, be1=be1,
        W2=W2, b2=b2, g2=g2, be2=be2,
        W3=W3, b3=b3, g3=g3, be3=be3,
        Wl=Wl, bl=bl,
    )
    out, _ = run(x, edge_index, edge_weights, weights, msg="f32")
    return out.astype(np.float32)


# revision 17
# speedup vs baseline: 2.5770x; 2.5770x over previous
"""GCN3 (3-layer GCNConv + BatchNorm + ReLU + linear head) on 8 Trainium2
NeuronCores.

Strategy (graph/data parallel, per sharding hint):
  - Nodes are sharded contiguously: core c owns destination nodes
    [c*12500, (c+1)*12500).
  - Per layer: each core computes z = r @ W_eff for its own nodes, the z
    shards are AllGathered (in 4 row-slices, giving 4 contiguous "striped"
    regions of 25000 rows so gather indices fit in int16), then each core
    gathers z rows for its incoming edges with dma_gather and scatter-adds
    them into per-supertile PSUM accumulators via one-hot matmuls
    (P^T[e, d] = norm_e * (dst_e == d), agg^T = msg^T @ P^T).
  - BatchNorm is folded into the next layer's weights:
    BN(r) @ W = r @ (diag(s) W) + 1 t'^T with a rank-1 per-destination
    correction dcoef[d] * t' added in PSUM (dcoef[d] = sum of incoming edge
    norms of d), where s/t come from an AllReduce of per-core stats.
"""

import math

import numpy as np

import concourse.bacc as bacc
import concourse.bass as bass
import concourse.mybir as mybir
import concourse.tile as tile
from concourse import library_config
from concourse.bass_utils import run_bass_kernel_spmd

# ---------------------------------------------------------------------------
# The walrus build in this container rejects the EVENT_SEMAPHORE_RANGE_CLEAR
# raw-ISA instruction Tile emits at context exit ("ISA wrong length").  Skip
# emitting it but keep the semaphore-ID bookkeeping; the kernel runs once per
# NEFF load so the end-of-kernel semaphore clear is not needed.
def _patched_clear(self, sems):
    if not sems:
        return
    sem_nums = [s.num if hasattr(s, "num") else s for s in sems]
    self._state.prepend_free_semaphores(sem_nums)
    for ps in self._tile_sem_poison_stack:
        ps.update(sem_nums)


bass.Bass.clear_and_free_semaphores = _patched_clear

EPS = 1e-5
NCORES = 8
KSLICES = 4  # AllGather row-slices -> striped regions addressable by int16
ST_W = 256  # supertile width (dst nodes per PSUM accumulator)
G_ST = 2  # supertiles per dma_gather call group
PT_B = 8  # P-matrix builds batched per DVE op
P = 128

DEBUG_SKIP = set()  # {"gather", "collectives"}

F32 = mybir.dt.float32
I16 = mybir.dt.int16


class _Cfg:
    def __init__(self, n_nodes, msg="f32"):
        assert n_nodes % (NCORES * KSLICES) == 0
        self.N = n_nodes
        self.shard = n_nodes // NCORES
        self.slice = self.shard // KSLICES
        self.region = self.slice * NCORES
        assert self.region <= 32767
        self.nst = math.ceil(self.shard / ST_W)
        self.last_w = self.shard - (self.nst - 1) * ST_W
        self.ngrp = math.ceil(self.nst / G_ST)
        self.npad = self.nst * ST_W  # padded shard width
        self.ntile = math.ceil(self.npad / P)  # 128-node tiles
        self.msg = msg
        self.msg_dt = F32 if msg == "f32" else mybir.dt.float16
        self.msg_np = np.float32 if msg == "f32" else np.float16


def _pack_idx16(flat_idx):
    """[n] int16 (n % 128 == 0) -> [128, n//16] with idx i at
    (partition i%16, slot i//16), replicated across the 8 groups of 16."""
    a = flat_idx.reshape(-1, 16).T  # [16, W]
    return np.tile(a, (8, 1)).astype(np.int16)


def _preprocess(cfg, x, edge_index, edge_weights):
    """Host-side graph preprocessing: normalization, edge partitioning, and
    the uniform (cross-core identical) gather/scatter schedule."""
    N = cfg.N
    row = np.asarray(edge_index[0], np.int64)
    col = np.asarray(edge_index[1], np.int64)
    ew = np.asarray(edge_weights, np.float32)

    deg = np.bincount(col, weights=ew.astype(np.float64), minlength=N).astype(
        np.float32
    ) + np.float32(1.0)
    dinv = (1.0 / np.sqrt(deg)).astype(np.float32)
    norm = (dinv[row] * ew * dinv[col]).astype(np.float32)

    arange = np.arange(N, dtype=np.int64)
    src = np.concatenate([row, arange])
    dst = np.concatenate([col, arange])
    nv = np.concatenate([norm, dinv * dinv]).astype(np.float32)

    dcoef = np.bincount(dst, weights=nv.astype(np.float64), minlength=N).astype(
        np.float32
    )

    core = dst // cfg.shard
    st = (dst % cfg.shard) // ST_W
    bkt = (src % cfg.shard) // cfg.slice
    sidx = (src // cfg.shard) * cfg.slice + (src % cfg.slice)
    dloc = (dst % cfg.shard) % ST_W

    key = ((core * cfg.nst) + st) * KSLICES + bkt
    order = np.argsort(key, kind="stable")
    sidx_s = sidx[order].astype(np.int16)
    dloc_s = dloc[order].astype(np.float32)
    nv_s = nv[order]

    counts = np.bincount(key, minlength=NCORES * cfg.nst * KSLICES).reshape(
        NCORES, cfg.nst, KSLICES
    )
    starts = np.zeros_like(counts)
    flat = counts.reshape(-1)
    starts.reshape(-1)[1:] = np.cumsum(flat)[:-1]

    # Uniform chunk counts: max over cores per (st, k).
    nck = np.ceil(counts / P).astype(np.int64).max(axis=0)  # [nst, K]

    # Build the call/chunk schedule (identical for every core).
    calls = []  # per (g, k): dict(num_idxs, w16_off, slot_off, chunks=[...])
    w16_off = 0
    slot_off = 0
    chunk_first = {}
    chunk_last = {}
    for s in range(cfg.nst):
        ks = [k for k in range(KSLICES) if nck[s, k] > 0]
        if ks:
            chunk_first[s] = (ks[0], 0)
            chunk_last[s] = (ks[-1], nck[s, ks[-1]] - 1)
    for g in range(cfg.ngrp):
        sts = list(range(g * G_ST, min((g + 1) * G_ST, cfg.nst)))
        for k in range(KSLICES):
            call_slots = int(sum(nck[s, k] for s in sts))
            if call_slots == 0:
                continue
            chunks = []
            loc = 0
            for s in sts:
                for j in range(int(nck[s, k])):
                    chunks.append(
                        dict(
                            st=s,
                            local_slot=loc,
                            gslot=slot_off + loc,
                            start=chunk_first[s] == (k, j),
                            stop=chunk_last[s] == (k, j),
                        )
                    )
                    loc += 1
            calls.append(
                dict(
                    g=g,
                    k=k,
                    slots=call_slots,
                    num_idxs=call_slots * P,
                    w16_off=w16_off,
                    slot_off=slot_off,
                    chunks=chunks,
                )
            )
            w16_off += call_slots * 8
            slot_off += call_slots
    sched = dict(calls=calls, tot_w16=w16_off, tot_slots=slot_off, nck=nck)

    # Per-core packed constant tensors.
    per_core = []
    for c in range(NCORES):
        idx_cols = []
        dst_cols = []
        nrm_cols = []
        for call in calls:
            k = call["k"]
            ns = call["slots"] * P
            fi = np.zeros(ns, np.int16)
            fd = np.zeros(ns, np.float32)
            fn = np.zeros(ns, np.float32)
            loc = 0
            sts = list(range(call["g"] * G_ST, min((call["g"] + 1) * G_ST, cfg.nst)))
            for s in sts:
                cnt = int(counts[c, s, k])
                off = int(starts[c, s, k])
                pos0 = loc * P
                fi[pos0 : pos0 + cnt] = sidx_s[off : off + cnt]
                fd[pos0 : pos0 + cnt] = dloc_s[off : off + cnt]
                fn[pos0 : pos0 + cnt] = nv_s[off : off + cnt]
                loc += int(nck[s, k])
            idx_cols.append(_pack_idx16(fi))
            dst_cols.append(fd.reshape(-1, P).T)
            nrm_cols.append(fn.reshape(-1, P).T)
        xs = np.asarray(x[c * cfg.shard : (c + 1) * cfg.shard], np.float32)
        xt = np.zeros((P, cfg.npad), np.float32)
        xt[:, : cfg.shard] = xs.T
        dc = np.zeros((1, cfg.npad), np.float32)
        dc[0, : cfg.shard] = dcoef[c * cfg.shard : (c + 1) * cfg.shard]
        per_core.append(
            dict(
                xt=xt,
                dcoef=dc,
                idx=np.concatenate(idx_cols, axis=1),
                dstv=np.concatenate(dst_cols, axis=1).astype(cfg.msg_np),
                ndst=-np.concatenate(dst_cols, axis=1).astype(np.float32),
                normv=np.concatenate(nrm_cols, axis=1).astype(cfg.msg_np),
            )
        )
    return sched, per_core


def _build_program(cfg, sched):
    nc = bacc.Bacc("TRN2", target_bir_lowering=False, debug=False, num_devices=NCORES, num_swdge_queues=4)
    dt = cfg.msg_dt
    calls = sched["calls"]
    max_slots = max(c["slots"] for c in calls)

    xt_d = nc.dram_tensor("xt", [P, cfg.npad], F32, kind="ExternalInput")
    idx_d = nc.dram_tensor("idx", [P, sched["tot_w16"]], I16, kind="ExternalInput")
    dstv_d = nc.dram_tensor("dstv", [P, sched["tot_slots"]], dt, kind="ExternalInput")
    ndst_d = nc.dram_tensor("ndst", [P, sched["tot_slots"]], F32, kind="ExternalInput")
    normv_d = nc.dram_tensor("normv", [P, sched["tot_slots"]], dt, kind="ExternalInput")
    iota_d = nc.dram_tensor("iota", [P, ST_W], dt, kind="ExternalInput")
    dcoef_d = nc.dram_tensor("dcoef", [1, cfg.npad], F32, kind="ExternalInput")
    w_d = [nc.dram_tensor(f"w{i}", [P, P], F32, kind="ExternalInput") for i in (1, 2, 3)]
    b_d = [nc.dram_tensor(f"b{i}", [P, 1], F32, kind="ExternalInput") for i in (1, 2, 3)]
    ga_d = [nc.dram_tensor(f"ga{i}", [P, 1], F32, kind="ExternalInput") for i in (1, 2, 3)]
    be_d = [nc.dram_tensor(f"be{i}", [P, 1], F32, kind="ExternalInput") for i in (1, 2, 3)]
    wl_d = nc.dram_tensor("wl", [P, 10], F32, kind="ExternalInput")
    blb_d = nc.dram_tensor("blb", [P, 10], F32, kind="ExternalInput")
    out_d = nc.dram_tensor("out", [cfg.shard, 10], F32, kind="ExternalOutput")

    ivn = 1.0 / float(cfg.N)

    with tile.TileContext(nc) as tc:
        with (
            tc.tile_pool(name="const", bufs=1) as cp,
            tc.tile_pool(name="rbuf", bufs=1) as rp,
            tc.tile_pool(name="msgs", bufs=6) as mp,
            tc.tile_pool(name="idxp", bufs=6) as ip,
            tc.tile_pool(name="ptp", bufs=4) as ptp,
            tc.tile_pool(name="zsb", bufs=4) as zp,
            tc.tile_pool(name="outp", bufs=4) as op_,
            tc.tile_pool(name="small", bufs=1) as sm,
            tc.tile_pool(name="psum", bufs=1, space="PSUM") as pp,
            tc.tile_pool(name="dram", bufs=1, space="DRAM") as dp,
        ):
            nc.gpsimd.load_library(library_config.mlp)

            # --- resident constants ---
            rT = rp.tile([P, cfg.npad], F32, tag="rT")  # x^T, later r^T (layer output)
            nc.sync.dma_start(rT[:], xt_d[:])
            dstv = cp.tile([P, sched["tot_slots"]], dt, tag="dstv")
            nc.sync.dma_start(dstv[:], dstv_d[:])
            normv = cp.tile([P, sched["tot_slots"]], dt, tag="normv")
            nc.sync.dma_start(normv[:], normv_d[:])
            ndst = cp.tile([P, sched["tot_slots"]], F32, tag="ndst")
            nc.sync.dma_start(ndst[:], ndst_d[:])
            iota = cp.tile([P, ST_W], dt, tag="iota")
            nc.sync.dma_start(iota[:], iota_d[:])
            dcoef = cp.tile([1, cfg.npad], F32, tag="dcoef")
            nc.sync.dma_start(dcoef[:], dcoef_d[:])
            w_sb, b_sb, ga_sb, be_sb = [], [], [], []
            for i in range(3):
                w = cp.tile([P, P], F32, name=f"w{i}", tag=f"w{i}")
                nc.sync.dma_start(w[:], w_d[i][:])
                w_sb.append(w)
                for lst, d in ((b_sb, b_d), (ga_sb, ga_d), (be_sb, be_d)):
                    t = cp.tile([P, 1], F32, name=f"p{i}_{len(lst)}", tag=f"p{i}_{len(lst)}")
                    nc.sync.dma_start(t[:], d[i][:])
                    lst.append(t)
            wl = cp.tile([P, 10], F32, tag="wl")
            nc.sync.dma_start(wl[:], wl_d[:])
            blb = cp.tile([P, 10], F32, tag="blb")
            nc.sync.dma_start(blb[:], blb_d[:])

            # --- per-layer DRAM buffers (reused across layers) ---
            z_local = dp.tile([cfg.shard, P], dt, name="z_local", tag="z_local")
            z_str_l = [
                [
                    dp.tile(
                        [cfg.region, P], dt,
                        addr_space="Local" if "collectives" in DEBUG_SKIP else "Shared",
                        name=f"z_str{l}_{k}", tag=f"z_str{l}_{k}",
                    )
                    for k in range(KSLICES)
                ]
                for l in range(3)
            ]

            w_rhs = w_sb[0]
            tp_sb = None  # t'^T [1, P] for the rank-1 BN-fold correction

            for layer in range(3):
                z_str = z_str_l[layer]
                # ---- z phase: z = r^T.T @ W_eff, shard-local ----
                for t in range(cfg.ntile):
                    rows = min(P, cfg.shard - t * P)
                    zp_ps = pp.tile([P, P], F32, tag="z", bufs=2, name=f"zp{layer}_{t}")
                    nc.tensor.matmul(
                        out=zp_ps[:],
                        lhsT=rT[:, t * P : (t + 1) * P],
                        rhs=w_rhs[:],
                        start=True,
                        stop=True,
                    )
                    zsb = zp.tile([P, P], dt, tag="zsb", name=f"zs{layer}_{t}")
                    nc.scalar.copy(zsb[:], zp_ps[:])
                    nc.sync.dma_start(
                        z_local[t * P : t * P + rows, :], zsb[:rows, :]
                    )
                # ---- AllGather z (4 row-slices -> striped regions) ----
                for k in range(KSLICES):
                    if "collectives" in DEBUG_SKIP:
                        for rr in range(NCORES):
                            nc.sync.dma_start(
                                z_str[k][rr * cfg.slice : (rr + 1) * cfg.slice, :],
                                z_local[k * cfg.slice : (k + 1) * cfg.slice, :],
                            )
                        continue
                    nc.sync.collective_compute(
                        "AllGather",
                        mybir.AluOpType.bypass,
                        replica_groups=[list(range(NCORES))],
                        ins=[z_local[k * cfg.slice : (k + 1) * cfg.slice, :]],
                        outs=[z_str[k][:]],
                    )
                # ---- gather + one-hot-matmul scatter ----
                agg = {}
                agg_done = set()
                chunk_i = 0
                for call_i, call in enumerate(calls):
                    g, k = call["g"], call["k"]
                    sts = list(range(g * G_ST, min((g + 1) * G_ST, cfg.nst)))
                    for s in sts:
                        if s not in agg:
                            agg[s] = pp.tile(
                                [P, ST_W], F32, tag="agg", bufs=6,
                                name=f"agg{layer}_{s}",
                            )
                    if "gather" in DEBUG_SKIP:
                        for s_ in sts:
                            if s_ in agg_done:
                                continue
                            agg_done.add(s_)
                            nc.tensor.matmul(
                                out=agg[s_][:], lhsT=iota[:, :P], rhs=iota[:],
                                start=True, stop=(layer == 0),
                            )
                        continue
                    w16 = call["slots"] * 8
                    idxt = ip.tile([P, w16], I16, tag="idx", name=f"ix{layer}_{g}_{k}")
                    nc.sync.dma_start(
                        idxt[:], idx_d[:, call["w16_off"] : call["w16_off"] + w16]
                    )
                    msgs = mp.tile(
                        [P, max_slots, P], dt, tag="msgs",
                        name=f"mg{layer}_{g}_{k}",
                    )[:, : call["slots"], :]
                    nc.gpsimd.dma_gather(
                        msgs[:], z_str[k][:], idxt[:],
                        call["num_idxs"], call["num_idxs"], P,
                        single_packet=False,
                        queue_num=call_i % 4,
                    )
                    # fold norm into the gathered messages (also zeroes the
                    # padding slots, keeping the one-hot P rows harmless)
                    nrm3 = normv[:, call["slot_off"] : call["slot_off"] + call["slots"]]
                    nc.vector.tensor_tensor(
                        out=msgs[:],
                        in0=msgs[:],
                        in1=nrm3.rearrange("p (s one) -> p s one", one=1).to_broadcast(
                            [P, call["slots"], P]
                        ),
                        op=mybir.AluOpType.mult,
                    )
                    chs = call["chunks"]
                    for b0 in range(0, len(chs), PT_B):
                        bw = min(PT_B, len(chs) - b0)
                        gs = chs[b0]["gslot"]
                        pt = ptp.tile(
                            [P, PT_B, ST_W], dt, tag="pt",
                            name=f"pt{layer}_{chunk_i}",
                        )
                        nc.vector.tensor_tensor(
                            out=pt[:, :bw, :],
                            in0=iota.rearrange("p (one w) -> p one w", one=1)
                            .to_broadcast([P, bw, ST_W]),
                            in1=dstv[:, gs : gs + bw]
                            .rearrange("p (s one) -> p s one", one=1)
                            .to_broadcast([P, bw, ST_W]),
                            op=mybir.AluOpType.is_equal,
                        )
                        for j in range(bw):
                            ch = chs[b0 + j]
                            stop = ch["stop"] and (layer == 0)
                            nc.tensor.matmul(
                                out=agg[ch["st"]][:],
                                lhsT=msgs[:, ch["local_slot"], :],
                                rhs=pt[:, j, :],
                                start=ch["start"],
                                stop=stop,
                            )
                        chunk_i += bw
                # ---- per-supertile epilogue ----
                ssum = sm.tile([P, cfg.nst], F32, name=f"ssum{layer}", tag=f"ssum{layer}")
                ssq = sm.tile([P, cfg.nst], F32, name=f"ssq{layer}", tag=f"ssq{layer}")
                for s in range(cfg.nst):
                    cols = ST_W if s < cfg.nst - 1 else cfg.last_w
                    if layer > 0:
                        nc.tensor.matmul(
                            out=agg[s][:],
                            lhsT=tp_sb[:],
                            rhs=dcoef[0:1, s * ST_W : s * ST_W + ST_W],
                            start=False,
                            stop=True,
                        )
                    if "no_accum" in DEBUG_SKIP:
                        nc.scalar.activation(
                            rT[:, s * ST_W : s * ST_W + cols],
                            agg[s][:, :cols],
                            mybir.ActivationFunctionType.Relu,
                            bias=b_sb[layer][:],
                        )
                        nc.vector.tensor_reduce(
                            ssum[:, s : s + 1],
                            rT[:, s * ST_W : s * ST_W + cols],
                            mybir.AxisListType.X,
                            mybir.AluOpType.add,
                        )
                    else:
                        nc.scalar.activation(
                            rT[:, s * ST_W : s * ST_W + cols],
                            agg[s][:, :cols],
                            mybir.ActivationFunctionType.Relu,
                            bias=b_sb[layer][:],
                            accum_out=ssum[:, s : s + 1],
                        )
                    scr = op_.tile([P, ST_W], dt, tag="scr", name=f"sc{layer}_{s}")
                    nc.scalar.activation(
                        scr[:, :cols],
                        rT[:, s * ST_W : s * ST_W + cols],
                        mybir.ActivationFunctionType.Square,
                        accum_out=ssq[:, s : s + 1],
                    )
                # ---- global BN stats ----
                spk = sm.tile([P, 2], F32, name=f"spk{layer}", tag=f"spk{layer}")
                nc.vector.tensor_reduce(
                    spk[:, 0:1], ssum[:], mybir.AxisListType.X, mybir.AluOpType.add
                )
                nc.vector.tensor_reduce(
                    spk[:, 1:2], ssq[:], mybir.AxisListType.X, mybir.AluOpType.add
                )
                st_in = dp.tile([P, 2], F32, name=f"sti{layer}", tag=f"sti{layer}")
                st_out = dp.tile([P, 2], F32, addr_space="Local" if "collectives" in DEBUG_SKIP else "Shared", name=f"sto{layer}", tag=f"sto{layer}")
                nc.sync.dma_start(st_in[:], spk[:])
                if "collectives" in DEBUG_SKIP:
                    nc.sync.dma_start(st_out[:], st_in[:])
                else:
                    nc.sync.collective_compute(
                        "AllReduce",
                        mybir.AluOpType.add,
                        replica_groups=[list(range(NCORES))],
                        ins=[st_in[:]],
                        outs=[st_out[:]],
                    )
                sg = sm.tile([P, 2], F32, name=f"sg{layer}", tag=f"sg{layer}")
                nc.sync.dma_start(sg[:], st_out[:])
                # s = gamma * rsqrt(var + eps); t = beta - mu * s
                mu = sm.tile([P, 1], F32, name=f"mu{layer}", tag=f"mu{layer}")
                nc.vector.tensor_scalar(
                    out=mu[:], in0=sg[:, 0:1], scalar1=ivn, scalar2=None,
                    op0=mybir.AluOpType.mult,
                )
                var = sm.tile([P, 1], F32, name=f"var{layer}", tag=f"var{layer}")
                nc.vector.tensor_scalar(
                    out=var[:], in0=sg[:, 1:2], scalar1=ivn, scalar2=None,
                    op0=mybir.AluOpType.mult,
                )
                mu2 = sm.tile([P, 1], F32, name=f"mu2{layer}", tag=f"mu2{layer}")
                nc.vector.tensor_tensor(
                    out=mu2[:], in0=mu[:], in1=mu[:], op=mybir.AluOpType.mult
                )
                nc.vector.tensor_tensor(
                    out=var[:], in0=var[:], in1=mu2[:], op=mybir.AluOpType.subtract
                )
                ve = sm.tile([P, 1], F32, name=f"ve{layer}", tag=f"ve{layer}")
                nc.vector.tensor_scalar(
                    out=ve[:], in0=var[:], scalar1=float(EPS), scalar2=None,
                    op0=mybir.AluOpType.add,
                )
                sd = sm.tile([P, 1], F32, name=f"sd{layer}", tag=f"sd{layer}")
                nc.scalar.activation(
                    sd[:], ve[:], mybir.ActivationFunctionType.Sqrt,
                )
                inv = sm.tile([P, 1], F32, name=f"inv{layer}", tag=f"inv{layer}")
                nc.vector.reciprocal(inv[:], sd[:])
                s_t = sm.tile([P, 1], F32, name=f"s{layer}", tag=f"s{layer}")
                nc.vector.tensor_tensor(
                    out=s_t[:], in0=ga_sb[layer][:], in1=inv[:],
                    op=mybir.AluOpType.mult,
                )
                t_t = sm.tile([P, 1], F32, name=f"t{layer}", tag=f"t{layer}")
                nc.vector.tensor_tensor(
                    out=t_t[:], in0=mu[:], in1=s_t[:], op=mybir.AluOpType.mult
                )
                nc.vector.tensor_tensor(
                    out=t_t[:], in0=be_sb[layer][:], in1=t_t[:],
                    op=mybir.AluOpType.subtract,
                )
                if layer < 2:
                    weff = sm.tile([P, P], F32, name=f"weff{layer}", tag=f"weff{layer}")
                    nc.vector.tensor_scalar(
                        out=weff[:], in0=w_sb[layer + 1][:], scalar1=s_t[:, 0:1],
                        scalar2=None, op0=mybir.AluOpType.mult,
                    )
                    w_rhs = weff
                    tp_ps = pp.tile([1, P], F32, tag="z", bufs=2, name=f"tpp{layer}")
                    nc.tensor.matmul(
                        out=tp_ps[:], lhsT=t_t[:], rhs=w_sb[layer + 1][:],
                        start=True, stop=True,
                    )
                    tp_sb = sm.tile([1, P], F32, name=f"tp{layer}", tag=f"tp{layer}")
                    nc.vector.tensor_copy(tp_sb[:], tp_ps[:])
                else:
                    s3, t3 = s_t, t_t

            # ---- classifier: out = (s3*r3 + t3)^T @ Wl + bl ----
            for t in range(cfg.ntile):
                rows = min(P, cfg.shard - t * P)
                h3 = op_.tile([P, P], F32, tag="h3", name=f"h3_{t}")
                nc.vector.tensor_scalar(
                    out=h3[:],
                    in0=rT[:, t * P : (t + 1) * P],
                    scalar1=s3[:, 0:1],
                    scalar2=t3[:, 0:1],
                    op0=mybir.AluOpType.mult,
                    op1=mybir.AluOpType.add,
                )
                po = pp.tile([P, 10], F32, tag="z", bufs=2, name=f"po{t}")
                nc.tensor.matmul(out=po[:], lhsT=h3[:], rhs=wl[:], start=True, stop=True)
                osb = op_.tile([P, 10], F32, tag="osb", name=f"ob{t}")
                nc.vector.tensor_tensor(
                    out=osb[:], in0=po[:], in1=blb[:], op=mybir.AluOpType.add
                )
                nc.sync.dma_start(out_d[t * P : t * P + rows, :], osb[:rows, :])
    nc.compile()
    return nc


_CACHE = {}


def _get_compiled(cfg, x, edge_index, edge_weights, weights):
    sched, per_core = _preprocess(cfg, x, edge_index, edge_weights)
    nc = _build_program(cfg, sched)
    iota = np.tile(np.arange(ST_W, dtype=cfg.msg_np), (P, 1))
    in_maps = []
    for c in range(NCORES):
        pc = per_core[c]
        m = dict(
            xt=pc["xt"],
            idx=pc["idx"],
            dstv=pc["dstv"],
            ndst=pc["ndst"],
            normv=pc["normv"],
            iota=iota,
            dcoef=pc["dcoef"],
            wl=np.asarray(weights["Wl"], np.float32),
            blb=np.tile(np.asarray(weights["bl"], np.float32), (P, 1)),
        )
        for i in (1, 2, 3):
            m[f"w{i}"] = np.asarray(weights[f"W{i}"], np.float32)
            m[f"b{i}"] = np.asarray(weights[f"b{i}"], np.float32).reshape(P, 1)
            m[f"ga{i}"] = np.asarray(weights[f"g{i}"], np.float32).reshape(P, 1)
            m[f"be{i}"] = np.asarray(weights[f"be{i}"], np.float32).reshape(P, 1)
        in_maps.append(m)
    return nc, in_maps


def run(x, edge_index, edge_weights, weights, msg="f32", trace=False):
    cfg = _Cfg(np.asarray(x).shape[0], msg=msg)
    nc, in_maps = _get_compiled(cfg, x, edge_index, edge_weights, weights)
    r = run_bass_kernel_spmd(nc, in_maps, list(range(NCORES)), trace=trace)
    out = np.concatenate([r.results[c]["out"] for c in range(NCORES)], axis=0)
    return out, r


def kernel(
    x,
    edge_index,
    edge_weights,
    W1, b1, g1, be1,
    W2, b2, g2, be2,
    W3, b3, g3, be3,
    Wl, bl,
):
    weights = dict(
        W1=W1, b1=b1, g1=g1, be1=be1,
        W2=W2, b2=b2, g2=g2, be2=be2,
        W3=W3, b3=b3, g3=g3, be3=be3,
        Wl=Wl, bl=bl,
    )
    out, _ = run(x, edge_index, edge_weights, weights, msg="f32")
    return out.astype(np.float32)
